# revision 11
# baseline (speedup 1.0000x reference)
"""Trainium2 Bass kernel for nn_Attention_25383256719981.

Dense transformer attention block:
  qkv = x @ W_qkv.T ; rotary(q,k,v) ; causal+padding-masked softmax(q k^T / sqrt(dh)) @ v ;
  out = heads @ W_out.T + b_out

Sharding: tensor-parallel over heads. 16 heads / 8 cores = 2 heads per core.
Each core computes its 2 heads' QKV projection, attention, and a partial
output projection (y_partial = O_heads @ W_out[:, head_cols].T); the host
sums the 8 f16 partials in f32 and adds b_out.

Key device-side design (v2, tuned from the NTFF profile):
  - dh dims stored DEINTERLEAVED ([evens|odds] per head, via host-side
    permutation of W_qkv rows / W_out cols / rotary tables), so the rotary
    pair-shuffle becomes dense half-block multiplies: 4 packed-f16 SBUF DVE
    ops per token block (eligible for the DVE 2x/4x perf modes).
  - cos/sin tables precomputed on host (f16), sin sign-folded in partner
    layout: rot(x) = x*cosd + swap_halves(x)*sind.
  - QKV PSUM tiles evacuated to SBUF f16 by ScalarE (batch 0) / Pool
    (batch 1, woven into attention while ScalarE runs exp).
  - Single QT/KT [128, N] per batch (head h occupies dh rows h*64:h*64+64);
    S matmuls run K=64 at base partition h*64 (same per-row cost as K=128).
  - S for both heads lands in one [128, 1024] 2-bank PSUM tile; ONE exp
    activation [128, 2, w] covers both heads; probabilities pt in f16 SBUF.
  - O^T accumulated per (head, qc) with an appended ones-row ([V|1]^T P^T)
    giving softmax row sums Z; 1/Z via DVE reciprocal straight on the PSUM
    row + gpsimd partition_broadcast (no DMA round trip).
  - Emission order software-pipelines the whole kernel to keep the PE
    continuously busy (DVFS ramp): [b0 QKV+transposes] [b0 attention with
    b1 QKV woven in] [b1 attention + projections].
  - y written as f16 (halves the partial-sum HBM traffic).
"""

import sys

import numpy as np

for _p in ("/opt/trn_rl_repo",):
    if _p not in sys.path:
        sys.path.insert(0, _p)

import concourse.bass as bass
import concourse.bacc as bacc
import concourse.mybir as mybir
import concourse.tile as tile
from concourse.bass_utils import run_bass_kernel_spmd
from concourse.masks import make_identity

# Problem shapes (hardcoded per contract).
B, N, D, H, DH = 2, 2048, 1024, 16, 64
NCORES = 8
HPC = H // NCORES            # heads per core
P = 128
NT = B * N                   # total tokens
SCALE = DH ** -0.5
FD = HPC * DH                # per-core features per tensor (128)
F3 = 3 * FD                  # 384
NEG = -1.0e30
NB = N // P                  # 16 token-blocks per batch
NCH = N // 512               # 4 x-chunks of 512 tokens per batch
KO = D // P                  # 8 contraction blocks

f32 = mybir.dt.float32
f16 = mybir.dt.float16
AF = mybir.ActivationFunctionType
ALU = mybir.AluOpType


def build_nc():
    nc = bacc.Bacc("TRN2", target_bir_lowering=False)

    xT = nc.dram_tensor("xT", [D, NT], f16, kind="ExternalInput")
    wqkvT = nc.dram_tensor("wqkvT", [D, F3], f16, kind="ExternalInput")
    woT = nc.dram_tensor("woT", [FD, D], f16, kind="ExternalInput")
    cosd = nc.dram_tensor("cosd", [N, DH], f16, kind="ExternalInput")
    sind = nc.dram_tensor("sind", [N, DH], f16, kind="ExternalInput")
    madd = nc.dram_tensor("madd", [P, B * NB], f32, kind="ExternalInput")
    caus = nc.dram_tensor("caus", [P, P], f16, kind="ExternalInput")
    y = nc.dram_tensor("y", [NT, D], f16, kind="ExternalOutput")

    xT_r = xT.rearrange("(ko p) t -> p ko t", p=P)          # [128, 8, 4096]
    wq_r = wqkvT.rearrange("(ko p) f -> p ko f", p=P)       # [128, 8, 384]
    cos_r = cosd.rearrange("(t p) d -> p t d", p=P)         # [128, 16, 64]
    sin_r = sind.rearrange("(t p) d -> p t d", p=P)

    with tile.TileContext(nc) as tc, \
            tc.tile_pool(name="const", bufs=1) as const, \
            tc.tile_pool(name="xp", bufs=2 * NCH) as xp, \
            tc.tile_pool(name="qsb", bufs=4) as qsb, \
            tc.tile_pool(name="tmpp", bufs=8) as tmpp, \
            tc.tile_pool(name="qkbp", bufs=4) as qkbp, \
            tc.tile_pool(name="vfp", bufs=2) as vfp, \
            tc.tile_pool(name="qtp", bufs=2) as qtp, \
            tc.tile_pool(name="ptp", bufs=4) as ptp, \
            tc.tile_pool(name="plp", bufs=2) as plp, \
            tc.tile_pool(name="zp", bufs=4) as zp, \
            tc.tile_pool(name="ysb", bufs=4) as ysb, \
            tc.tile_pool(name="psS", bufs=2, space="PSUM") as psS, \
            tc.tile_pool(name="psM", bufs=2, space="PSUM") as psM, \
            tc.tile_pool(name="psO", bufs=2, space="PSUM") as psO:

        # ---- constants / weights (x chunks first: QKV needs them first) ---
        w_sb = const.tile([P, KO, F3], f16, tag="w")
        nc.sync.dma_start(w_sb[:, :, :], wq_r)
        x_sbs = {}
        for b in range(B):
            for c in range(NCH):
                x_sb = xp.tile([P, KO, 512], f16, tag="x", name=f"x_{b}_{c}")
                tok0 = b * N + c * 512
                nc.sync.dma_start(x_sb[:, :, :], xT_r[:, :, tok0:tok0 + 512])
                x_sbs[(b, c)] = x_sb
        wo_sb = const.tile([FD, D], f16, tag="wo")
        nc.sync.dma_start(wo_sb[:, :], woT[:, :])
        caus01 = const.tile([P, P], f16, tag="caus01")
        nc.sync.dma_start(caus01[:, :], caus[:, :])
        madd_sb = const.tile([P, B * NB], f32, tag="madd")
        nc.sync.dma_start(madd_sb[:, :], madd[:, :])
        cos_sb = const.tile([P, NB, DH], f16, tag="cos")
        nc.sync.dma_start(cos_sb[:, :, :], cos_r)
        sin_sb = const.tile([P, NB, DH], f16, tag="sin")
        nc.sync.dma_start(sin_sb[:, :, :], sin_r)
        ident = const.tile([P, P], f16, tag="ident")
        make_identity(nc, ident)
        onecol = const.tile([P, 1], f32, tag="onecol")
        nc.gpsimd.memset(onecol, 1.0)

        # Per-batch persistent tiles.
        QT = {}
        KT = {}
        VF = {}
        for b in range(B):
            QT[b] = qtp.tile([P, N], f16, tag="QT", name=f"QT{b}")
            KT[b] = qtp.tile([P, N], f16, tag="KT", name=f"KT{b}")
            VF[b] = vfp.tile([P, NB, HPC * (DH + 1)], f16, tag="vf",
                             name=f"VF{b}")

        def emit_vf_ones(b):
            nc.vector.tensor_copy(
                VF[b][:, :, DH::DH + 1],
                onecol[:, None, :].to_broadcast([P, NB, HPC]))

        def ecopy(eng, out, in_):
            if eng is nc.scalar:
                eng.copy(out, in_)
            else:
                eng.tensor_copy(out, in_)

        # ---------------- QKV + rotary + transpose, one token block -------
        # evac_eng: engine for the PSUM->SBUF f16 evacuation.
        # copy_engs: engines for the two transpose-result copies.
        def emit_qkv_block(b, t, evac_eng, copy_engs):
            c, tb = divmod(t, 4)
            x_sb = x_sbs[(b, c)]
            qkv_ps = psM.tile([P, 512], f32, tag="mm", name=f"qkv_{b}_{t}")
            for ko in range(KO):
                nc.tensor.matmul(
                    qkv_ps[:, 0:F3],
                    x_sb[:, ko, tb * P:(tb + 1) * P],
                    w_sb[:, ko, :],
                    start=(ko == 0), stop=(ko == KO - 1),
                )
            qkv_sb = qsb.tile([P, F3], f16, tag="qs", name=f"qsb_{b}_{t}")
            ecopy(evac_eng, qkv_sb, qkv_ps[:, 0:F3])
            # rotary: out = x*cos + swap_halves(x)*sind  (dense f16 SBUF ops)
            g6 = qkv_sb.rearrange("p (g d) -> p g d", g=6)
            tmp = tmpp.tile([P, F3], f16, tag="tmp", name=f"tmp_{b}_{t}")
            t6 = tmp.rearrange("p (g d) -> p g d", g=6)
            se = sin_sb[:, t, 0:32][:, None, :].to_broadcast([P, 6, 32])
            so = sin_sb[:, t, 32:64][:, None, :].to_broadcast([P, 6, 32])
            nc.vector.tensor_tensor(t6[:, :, 0:32], g6[:, :, 32:64], se, ALU.mult)
            nc.vector.tensor_tensor(t6[:, :, 32:64], g6[:, :, 0:32], so, ALU.mult)
            cq = tmpp.tile([P, F3], f16, tag="cq", name=f"cq_{b}_{t}")
            c6 = cq.rearrange("p (g d) -> p g d", g=6)
            cb = cos_sb[:, t, :][:, None, :].to_broadcast([P, 6, DH])
            nc.vector.tensor_tensor(c6, g6, cb, ALU.mult)
            qkb = qkbp.tile([P, 2 * FD], f16, tag="qkb", name=f"qkb_{b}_{t}")
            nc.vector.tensor_tensor(qkb, tmp[:, 0:2 * FD], cq[:, 0:2 * FD],
                                    ALU.add)
            vf_v = VF[b][:, t, :].rearrange("p (h c) -> p h c", h=HPC)[:, :, 0:DH]
            tmp_v = tmp[:, 2 * FD:F3].rearrange("p (h d) -> p h d", h=HPC)
            cq_v = cq[:, 2 * FD:F3].rearrange("p (h d) -> p h d", h=HPC)
            nc.vector.tensor_tensor(vf_v, tmp_v, cq_v, ALU.add)
            # transposes q-pair and k-pair -> QT/KT columns
            for which, dst in ((0, QT[b]), (1, KT[b])):
                tr_ps = psM.tile([P, P], f16, tag="mm", name=f"tr{which}_{b}_{t}")
                nc.tensor.transpose(tr_ps, qkb[:, which * P:(which + 1) * P],
                                    ident)
                copy_engs[which].tensor_copy(dst[:, t * P:(t + 1) * P], tr_ps)

        # ---------------- attention -----------------------------------
        # Per batch: 40 (qc, kb) pairs; S pair -> exp -> (lag 2) O pair.
        def attn_pairs(b):
            return [(qc, kb) for qc in range(NCH) for kb in range(4 * qc + 4)]

        def emit_S(b, qc, kb):
            qs = max(kb * P, 512 * qc)
            off = qs - 512 * qc
            w = 512 - off
            S_t = psS.tile([P, 1024], f32, tag="s", name=f"S_{b}_{qc}_{kb}")
            for h in range(HPC):
                nc.tensor.matmul(
                    S_t[:, h * 512:h * 512 + w],
                    KT[b][h * DH:(h + 1) * DH, kb * P:(kb + 1) * P],
                    QT[b][h * DH:(h + 1) * DH, qs:qs + w],
                    start=True, stop=True)
            return (b, qc, kb, off, w, S_t)

        def emit_exp(b, qc, kb, off, w, S_t):
            pt = ptp.tile([P, 1024], f16, tag="pt", name=f"pt_{b}_{qc}_{kb}")
            col = b * NB + kb
            sv = S_t.rearrange("p (h w) -> p h w", h=2)[:, :, 0:w]
            pv = pt.rearrange("p (h w) -> p h w", h=2)[:, :, 0:w]
            nc.scalar.activation(pv, sv, AF.Exp,
                                 bias=madd_sb[:, col:col + 1], scale=SCALE)
            if kb >= 4 * qc:  # chunk starts at the diagonal block
                cv = pt.rearrange("p (h w) -> p h w", h=2)[:, :, 0:P]
                nc.gpsimd.tensor_tensor(
                    cv, cv, caus01[:, None, :].to_broadcast([P, 2, P]),
                    ALU.mult)
            return (b, qc, kb, off, w, pt)

        O_tiles = {}
        PL_tiles = {}
        proj_ready = []          # qc's whose PL is complete (per batch)

        def emit_O(b, qc, kb, off, w, pt):
            for h in range(HPC):
                if kb == 0:
                    O_tiles[(b, h, qc)] = psO.tile(
                        [DH + 1, 512], f32, tag="o", name=f"O_{b}_{h}_{qc}")
                O_ps = O_tiles[(b, h, qc)]
                nc.tensor.matmul(
                    O_ps[:, off:512],
                    VF[b][:, kb, h * (DH + 1):(h + 1) * (DH + 1)],
                    pt[:, h * 512:h * 512 + w],
                    start=(kb == 0), stop=(kb == 4 * qc + 3),
                )
            if kb == 4 * qc + 3:
                PLq = plp.tile([P, 512], f16, tag="PL", name=f"PL_{b}_{qc}")
                PL_tiles[(b, qc)] = PLq
                for h in range(HPC):
                    O_ps = O_tiles.pop((b, h, qc))
                    # read Z row + raw O out of PSUM promptly (frees the bank
                    # without waiting on the reciprocal/broadcast chain)
                    zinv = zp.tile([1, 512], f32, tag="zi",
                                   name=f"zi_{b}_{h}_{qc}")
                    nc.vector.reciprocal(zinv, O_ps[DH:DH + 1, :])
                    o_sb = zp.tile([DH, 512], f16, tag="osb",
                                   name=f"osb_{b}_{h}_{qc}")
                    nc.vector.tensor_copy(o_sb, O_ps[0:DH, :])
                    rb = zp.tile([DH, 512], f32, tag="rb",
                                 name=f"rb_{b}_{h}_{qc}")
                    nc.gpsimd.partition_broadcast(rb, zinv)
                    nc.gpsimd.tensor_tensor(
                        PLq[h * DH:(h + 1) * DH, :], o_sb, rb, ALU.mult)
                proj_ready.append((b, qc))

        # Output projection for one 128-token block (2 matmuls + evac + DMA)
        def emit_proj_tb(b, qc, tb, evac_engs):
            PLq = PL_tiles[(b, qc)]
            t = qc * 4 + tb
            y_sb = ysb.tile([P, D], f16, tag="ysb", name=f"y_{b}_{t}")
            for dc in range(2):
                y_ps = psM.tile([P, 512], f32, tag="mm", name=f"yps_{b}_{t}_{dc}")
                nc.tensor.matmul(y_ps,
                                 PLq[:, tb * P:(tb + 1) * P],
                                 wo_sb[:, dc * 512:(dc + 1) * 512],
                                 start=True, stop=True)
                ecopy(evac_engs[dc], y_sb[:, dc * 512:(dc + 1) * 512], y_ps)
            r0 = b * N + t * P
            nc.sync.dma_start(y[r0:r0 + P, :], y_sb)
            if tb == 3:
                del PL_tiles[(b, qc)]

        # ---------------- emission schedule ----------------------------
        emit_vf_ones(0)
        # Phase 1: b0 QKV + transposes (evac on Scalar -- idle here; copies DVE)
        for t in range(NB):
            emit_qkv_block(0, t, nc.scalar, (nc.vector, nc.vector))
        emit_vf_ones(1)

        # Phase 2: b0 attention with b1 QKV blocks woven in (evac on Pool,
        # copies Pool/DVE to keep ScalarE free for exp). Also drain b0
        # projections as PL chunks complete.
        pairs0 = attn_pairs(0)
        b1_blocks = list(range(NB))
        # weave positions: spread 16 blocks over 40 pairs
        weave_after = {int(round((i + 1) * len(pairs0) / (NB + 1))): i
                       for i in range(NB)}
        pend = []                # exp'd units awaiting O (lag 2)
        proj_tb_queue = []       # (b, qc, tb) pending projection blocks

        def drain_proj(kmax=1):
            for _ in range(kmax):
                if proj_ready:
                    b, qc = proj_ready[0]
                    done = [tb for (bb, qq, tb) in proj_tb_queue
                            if (bb, qq) == (b, qc)]
                    nxt = len(done)
                    proj_tb_queue.append((b, qc, nxt))
                    # b0 projections run while ScalarE's exp load is lighter
                    # than DVE's (rotary weave); b1 projections run in phase 3
                    # where DVE is the lighter engine.
                    eng = nc.scalar if b == 0 else nc.vector
                    emit_proj_tb(b, qc, nxt, (eng, eng))
                    if nxt == 3:
                        proj_ready.pop(0)

        def push_unit(u, lag=2):
            pend.append(u)
            if len(pend) > lag:
                emit_O(*pend.pop(0))
                drain_proj(1)

        for i, (qc, kb) in enumerate(pairs0):
            su = emit_S(0, qc, kb)
            push_unit(emit_exp(*su))
            if i in weave_after:
                emit_qkv_block(1, weave_after[i], nc.vector,
                               (nc.vector, nc.vector))
        while pend:
            emit_O(*pend.pop(0))
            drain_proj(1)

        # Phase 3: b1 attention (+ remaining projections woven)
        for i, (qc, kb) in enumerate(pairs0):
            su = emit_S(1, qc, kb)
            push_unit(emit_exp(*su))
            drain_proj(1)
        while pend:
            emit_O(*pend.pop(0))
            drain_proj(2)
        while proj_ready:
            drain_proj(2)

    nc.compile()
    return nc


_PERM = np.concatenate([np.arange(0, DH, 2), np.arange(1, DH, 2)])  # deint


def prep_inputs(x, mask, rotary_pos_emb, W_qkv, W_out, dt_mode="f16"):
    """Host-side shard prep: per-core input dicts (layout/permutation only,
    plus mask encode and rotary cos/sin table build)."""
    x = np.asarray(x, dtype=np.float32)
    W_qkv = np.asarray(W_qkv, dtype=np.float32)
    W_out = np.asarray(W_out, dtype=np.float32)
    rope = np.asarray(rotary_pos_emb, dtype=np.float32)
    mask = np.asarray(mask)

    xT = np.ascontiguousarray(x.reshape(NT, D).T.astype(np.float16))
    madd = np.where(mask, np.float32(0.0), np.float32(NEG)).astype(np.float32)
    madd_dev = np.ascontiguousarray(
        madd.reshape(B, NB, P).transpose(2, 0, 1).reshape(P, B * NB))
    kidx = np.arange(P)[:, None]
    qidx = np.arange(P)[None, :]
    caus = (qidx >= kidx).astype(np.float16)          # 0/1 multiplicative
    freq = rope[-N:, :]                               # [N, 64]
    cosd = np.cos(freq)[:, _PERM].astype(np.float16)
    # sind partner layout: slot [0:32] = -sin(even freqs), [32:64] = +sin(odd)
    sind = np.concatenate([-np.sin(freq[:, 0::2]), np.sin(freq[:, 1::2])],
                          axis=1).astype(np.float16)
    cosd = np.ascontiguousarray(cosd)
    sind = np.ascontiguousarray(sind)

    in_maps = []
    for c in range(NCORES):
        rows = []
        for tsel in range(3):                      # q, k, v row blocks
            for h in (HPC * c, HPC * c + 1):
                o = tsel * H * DH + h * DH
                rows.append(W_qkv[o + _PERM, :])
        wqkvT = np.ascontiguousarray(
            np.concatenate(rows, axis=0).T.astype(np.float16))
        cols = np.concatenate([FD * c + h * DH + _PERM for h in range(HPC)])
        woT = np.ascontiguousarray(W_out[:, cols].T.astype(np.float16))
        in_maps.append({
            "xT": xT, "wqkvT": wqkvT, "woT": woT,
            "cosd": cosd, "sind": sind, "madd": madd_dev, "caus": caus,
        })
    return in_maps


def _ensure_ntff_hook():
    """Install antenv.axon_hooks + the ctypes NTFF profile hook if the image
    lacks them (needed only for trace=True timing runs, not for kernel())."""
    import types
    try:
        from antenv.axon_hooks import get_axon_ntff_profile_hook  # noqa: F401
        return
    except ImportError:
        pass
    try:
        import antenv
        mod = types.ModuleType("antenv.axon_hooks")
        _state = {"hook": None}

        def set_axon_ntff_profile_hook(h):
            _state["hook"] = h

        def get_axon_ntff_profile_hook():
            return _state["hook"]

        mod.set_axon_ntff_profile_hook = set_axon_ntff_profile_hook
        mod.get_axon_ntff_profile_hook = get_axon_ntff_profile_hook
        sys.modules["antenv.axon_hooks"] = mod
        antenv.axon_hooks = mod
        from trn_agent_boot.trn_boot import _ntff_profile_via_ctypes
        hook = _ntff_profile_via_ctypes("/opt/axon/libaxon_pjrt.so")
        if hook is not None:
            set_axon_ntff_profile_hook(hook)
    except Exception as e:  # degrade to untimed runs
        print(f"ntff hook install failed: {e!r}", file=sys.stderr)


_NC_CACHE = {}


def _get_nc(dt_mode="f16"):
    if "nc" not in _NC_CACHE:
        _NC_CACHE["nc"] = build_nc()
    return _NC_CACHE["nc"]


def run_cores(in_maps, trace=False, dt_mode="f16"):
    if trace:
        _ensure_ntff_hook()
    nc = _get_nc(dt_mode)
    res = run_bass_kernel_spmd(
        nc, in_maps, core_ids=list(range(NCORES)), trace=trace,
        trace_cores=list(range(NCORES)) if trace else None,
    )
    return res


DT_MODE = "f16"


def kernel(x, mask, rotary_pos_emb, W_qkv, W_out, b_out):
    in_maps = prep_inputs(x, mask, rotary_pos_emb, W_qkv, W_out)
    res = run_cores(in_maps, trace=False)
    y = np.zeros((NT, D), dtype=np.float32)
    for r in res.results:
        y += r["y"]
    y += np.asarray(b_out, dtype=np.float32)[None, :]
    return y.reshape(B, N, D)
